# revision 1
# baseline (speedup 1.0000x reference)
"""Segment-mean reduction (grouped mean over sorted segment ids) on 8 trn2 cores.

Strategy (data-parallel over batch): each core handles one batch row.
out[g, :] = mean over rows s of feats with segment_ids[s] == g.

Host-side staging (inside kernel(), before upload):
  * The 1024 groups are split into 8 chunks of 128 groups. Rows of feats are
    reordered per core so that each chunk's rows are contiguous and padded to a
    multiple of 128; every 128-row tile then belongs to exactly ONE chunk, and
    the tile->chunk map is identical across cores (chunk tile counts are the
    max over cores). Pad rows point at row 0 with local id -1 (never matches).
  * feats are shipped as an fp16 hi/lo split (hi = fp16(x), lo = fp16(x - hi)),
    packed per tile as [128, 512] = [hi || lo]. fp16 streams the PE at full
    rate (fp32 is 1/4 rate); adding the hi and lo halves of the 512-wide
    matmul output recovers ~fp32 accuracy with ONE matmul per tile.

Device program per tile t (static schedule):
    onehot[s, g] = (iota[g] == sl[s])     # one tensor_scalar, DVE/GpSimd split
    psum[chunk(t)][:, 0:512] += onehot.T @ (hi || lo)   # PE, fp16 -> fp32 PSUM
and per chunk c at the end:
    sum = psum[:, :256] + psum[:, 256:]   # DVE
    out rows = sum * recip_count          # ACT copy with per-partition scale
then DMA to DRAM.

Per-core HBM traffic ~= feats (8 MB) + out (1 MB) => memory-bound.
"""

import numpy as np

import concourse.bass as bass
import concourse.bacc as bacc
import concourse.mybir as mybir
import concourse.tile as tile
from concourse.bass_utils import run_bass_kernel_spmd

F32 = mybir.dt.float32
F16 = mybir.dt.float16
P = 128  # partitions


def _host_layout(seg_all: np.ndarray, G: int):
    """Chunk-aligned row layout: shared tile->chunk map, per-core gather
    indices and aux arrays."""
    R, S = seg_all.shape
    CH = G // P

    chunk_of = seg_all // P  # [R, S]
    cnt = np.stack([np.bincount(chunk_of[r], minlength=CH) for r in range(R)])
    tiles_per_chunk = (cnt.max(axis=0) + P - 1) // P  # [CH]
    T = int(tiles_per_chunk.sum())

    chunk_of_tile = np.repeat(np.arange(CH), tiles_per_chunk)  # [T]
    first = np.full(CH, -1, np.int64)
    last = np.full(CH, -1, np.int64)
    for i, c in enumerate(chunk_of_tile):
        if first[c] < 0:
            first[c] = i
        last[c] = i

    Spad = T * P
    gather = np.zeros((R, Spad), np.int64)
    sl = np.full((R, Spad), -1.0, np.float32)  # local group id, -1 for pads
    for r in range(R):
        pos = 0
        for c in range(CH):
            rows = np.nonzero(chunk_of[r] == c)[0]
            n = len(rows)
            tc = int(tiles_per_chunk[c])
            gather[r, pos:pos + n] = rows
            sl[r, pos:pos + n] = (seg_all[r, rows] % P).astype(np.float32)
            pos += tc * P
    # aux arrays in [P, T] tile layout: column t, partition p <- padded row t*P+p
    aux_sl = np.ascontiguousarray(sl.reshape(R, T, P).transpose(0, 2, 1))
    # per-group reciprocal counts, [P, CH]: partition p, col c -> group c*P+p
    counts = np.stack(
        [np.bincount(seg_all[r], minlength=G) for r in range(R)]
    ).astype(np.float32)
    recip = (1.0 / np.maximum(counts, 1.0)).reshape(R, CH, P)
    aux_rc = np.ascontiguousarray(recip.transpose(0, 2, 1))

    return dict(T=T, CH=CH, chunk_of_tile=chunk_of_tile, first=first, last=last,
                gather=gather, aux_sl=aux_sl, aux_rc=aux_rc)


def _build_program(H: int, G: int, lay, grp: int = 8):
    T, CH = lay["T"], lay["CH"]
    chunk_of_tile = lay["chunk_of_tile"]
    first, last = lay["first"], lay["last"]
    H2 = 2 * H  # hi || lo

    nc = bacc.Bacc("TRN2", target_bir_lowering=False, debug=False, num_devices=8)
    hl_d = nc.dram_tensor("feats_hl", [T * P, H2], F16, kind="ExternalInput")
    sl_d = nc.dram_tensor("aux_sl", [P, T], F32, kind="ExternalInput")
    rc_d = nc.dram_tensor("aux_rc", [P, CH], F32, kind="ExternalInput")
    iota_d = nc.dram_tensor("iota", [P, P], F16, kind="ExternalInput")
    out_d = nc.dram_tensor("out", [G, H], F32, kind="ExternalOutput")

    with tile.TileContext(nc) as tc:
        with (
            tc.tile_pool(name="const", bufs=1) as constp,
            tc.tile_pool(name="feats", bufs=3) as fpool,
            tc.tile_pool(name="mt", bufs=8) as mtpool,
            tc.tile_pool(name="outp", bufs=2) as opool,
            tc.tile_pool(name="psum", bufs=1, space="PSUM") as pp,
        ):
            iota_t = constp.tile([P, P], F16, tag="iota")
            nc.sync.dma_start(iota_t[:], iota_d.ap())
            sl_t = constp.tile([P, T], F32, tag="sl")
            nc.sync.dma_start(sl_t[:], sl_d.ap())
            rc_t = constp.tile([P, CH], F32, tag="rc")
            nc.sync.dma_start(rc_t[:], rc_d.ap())

            psum_tiles = [
                pp.tile([P, H2], F32, tag=f"ps{c}", name=f"ps{c}") for c in range(CH)
            ]

            hl_v = hl_d.ap().rearrange("(a p) h -> p a h", p=P)

            ngrp = (T + grp - 1) // grp
            for g0 in range(ngrp):
                t0 = g0 * grp
                nt = min(grp, T - t0)
                ft = fpool.tile([P, grp, H2], F16, tag="ft")
                nc.sync.dma_start(ft[:, :nt, :], hl_v[:, t0:t0 + nt, :])
                for tt in range(nt):
                    t = t0 + tt
                    c = int(chunk_of_tile[t])
                    mt = mtpool.tile([P, P], F16, tag="mt", name="mt")
                    # onehot[s, g] = (iota[g] == sl[s]) on DVE (GpSimd is ~8x
                    # slower for this op and port-shares with DVE)
                    nc.vector.tensor_scalar(
                        mt[:],
                        iota_t[:],
                        sl_t[:, t:t + 1],
                        None,
                        mybir.AluOpType.is_equal,
                    )
                    nc.tensor.matmul(
                        psum_tiles[c][:], mt[:], ft[:, tt, :],
                        start=(t == first[c]), stop=(t == last[c]),
                    )

            for c in range(CH):
                st = opool.tile([P, H], F32, tag="st", name="st")
                ot = opool.tile([P, H], F32, tag="ot", name="ot")
                if first[c] >= 0:
                    # st = psum_hi * (1/count) on ACT (single PSUM operand)
                    nc.scalar.activation(
                        st[:], psum_tiles[c][:, :H],
                        mybir.ActivationFunctionType.Copy,
                        scale=rc_t[:, c:c + 1],
                    )
                    # ot = (psum_lo * (1/count)) + st on DVE
                    nc.vector.scalar_tensor_tensor(
                        ot[:], psum_tiles[c][:, H:], rc_t[:, c:c + 1], st[:],
                        mybir.AluOpType.mult, mybir.AluOpType.add,
                    )
                else:
                    nc.vector.memset(ot[:], 0.0)
                nc.sync.dma_start(out_d.ap()[c * P:(c + 1) * P, :], ot[:])

    nc.compile()
    return nc


def kernel(feats, segment_ids, num_groups, _trace=False):
    feats = np.ascontiguousarray(np.asarray(feats, dtype=np.float32))
    seg_all = np.ascontiguousarray(np.asarray(segment_ids, dtype=np.int32))
    G = int(num_groups)
    B, S, H = feats.shape
    assert seg_all.shape == (B, S) and B == 8 and G % P == 0

    lay = _host_layout(seg_all, G)
    nc = _build_program(H, G, lay)

    iota_arr = np.broadcast_to(
        np.arange(P, dtype=np.float16)[None, :], (P, P)
    ).copy()

    in_maps = []
    for r in range(B):
        fr = feats[r][lay["gather"][r]]  # [Spad, H] fp32, chunk-aligned
        hi = fr.astype(np.float16)
        lo = (fr - hi.astype(np.float32)).astype(np.float16)
        hl = np.concatenate([hi, lo], axis=1)  # [Spad, 2H]
        in_maps.append({
            "feats_hl": hl,
            "aux_sl": lay["aux_sl"][r],
            "aux_rc": lay["aux_rc"][r],
            "iota": iota_arr,
        })
    res = run_bass_kernel_spmd(nc, in_maps, list(range(B)), trace=_trace)
    out = np.stack([res.results[r]["out"] for r in range(B)])
    if _trace:
        return out, res
    return out



# revision 6
# speedup vs baseline: 1.0777x; 1.0777x over previous
"""Segment-mean reduction (grouped mean over sorted segment ids) on 8 trn2 cores.

Strategy (data-parallel over batch): each core handles one batch row.
out[g, :] = mean over rows s of feats with segment_ids[s] == g.

Host-side staging (inside kernel(), before upload):
  * The 1024 groups are split into 8 chunks of 128 groups. Rows of feats are
    reordered per core so that each chunk's rows are contiguous and padded to a
    multiple of 128; every 128-row tile then belongs to exactly ONE chunk, and
    the tile->chunk map is identical across cores (chunk tile counts are the
    max over cores). Pad rows point at row 0 with local id -1 (never matches).
  * feats are shipped as an fp16 hi/lo split (hi = fp16(x), lo = fp16(x - hi)),
    packed per tile as [128, 512] = [hi || lo]; adding the hi and lo halves of
    the 512-wide matmul output recovers ~fp32 accuracy with ONE matmul/tile.
  * DRAM layout is transposed to [128, T*512] so every DMA partition line is a
    large contiguous run (1 KiB per tile per partition; tiles of one group are
    adjacent) instead of the 1 KiB strided gather of the [T*128, 512] layout.

Device program (static schedule):
  * consts (iota / sl / rc) stream in on the Activation HWDGE ring; feats
    groups stream in on the SP ring (skewed sizes: small first groups so the
    first matmul starts early, then 8-tile groups).
  * ~10 dummy matmuls on garbage SBUF warm the PE HAM clock gate (cold PE runs
    at 1.2 GHz; warm at 2.4 GHz) during the engine-bringup/first-DMA window.
    They write psum[7], which the first real accumulation overwrites
    (start=True).
  * per tile t: onehot[s, g] = (iota[g] == sl[s]) on DVE (all-fp16), then
    psum[chunk(t)][:, 0:512] += onehot.T @ (hi || lo) on PE.
  * per chunk c, emitted right after chunk c's last tile: st = psum_hi * rc on
    ACT, ot = psum_lo * rc + st on DVE, then out-row DMA on the Activation
    ring (so it never queues behind feats triggers on SP).

Accumulation order per chunk, the hi/lo split, and the finish math are
IDENTICAL to the reference baseline kernel => bitwise-equal outputs.

Per-core HBM traffic ~= feats (9.4 MB padded) + out (1 MB) => memory-bound;
roofline ~29 us at the ~358 GB/s per-core HBM limit.
"""

import numpy as np

import concourse.bass as bass
import concourse.bacc as bacc
import concourse.mybir as mybir
import concourse.tile as tile
from concourse.bass_utils import run_bass_kernel_spmd

F32 = mybir.dt.float32
F16 = mybir.dt.float16
P = 128  # partitions
H2 = 512  # hi || lo columns per tile
NDUMMY = 10  # PE warmup matmuls


def _host_layout(seg_all: np.ndarray, G: int):
    """Chunk-aligned row layout: shared tile->chunk map, per-core gather
    indices and aux arrays."""
    R, S = seg_all.shape
    CH = G // P

    chunk_of = seg_all // P  # [R, S]
    cnt = np.stack([np.bincount(chunk_of[r], minlength=CH) for r in range(R)])
    tiles_per_chunk = (cnt.max(axis=0) + P - 1) // P  # [CH]
    T = int(tiles_per_chunk.sum())

    chunk_of_tile = np.repeat(np.arange(CH), tiles_per_chunk)  # [T]
    first = np.full(CH, -1, np.int64)
    last = np.full(CH, -1, np.int64)
    for i, c in enumerate(chunk_of_tile):
        if first[c] < 0:
            first[c] = i
        last[c] = i

    Spad = T * P
    gather = np.zeros((R, Spad), np.int64)
    sl = np.full((R, Spad), -1.0, np.float32)  # local group id, -1 for pads
    for r in range(R):
        pos = 0
        for c in range(CH):
            rows = np.nonzero(chunk_of[r] == c)[0]
            n = len(rows)
            tc = int(tiles_per_chunk[c])
            gather[r, pos:pos + n] = rows
            sl[r, pos:pos + n] = (seg_all[r, rows] % P).astype(np.float32)
            pos += tc * P
    # aux arrays in [P, T] tile layout: column t, partition p <- padded row t*P+p
    aux_sl = np.ascontiguousarray(sl.reshape(R, T, P).transpose(0, 2, 1))
    # per-group reciprocal counts, [P, CH]: partition p, col c -> group c*P+p
    counts = np.stack(
        [np.bincount(seg_all[r], minlength=G) for r in range(R)]
    ).astype(np.float32)
    recip = (1.0 / np.maximum(counts, 1.0)).reshape(R, CH, P)
    aux_rc = np.ascontiguousarray(recip.transpose(0, 2, 1))

    return dict(T=T, CH=CH, chunk_of_tile=chunk_of_tile, first=first, last=last,
                gather=gather, aux_sl=aux_sl, aux_rc=aux_rc)


def _dma_groups(T: int):
    """Skewed group sizes: small first so the first matmul starts early."""
    sizes = []
    for s in (2, 2, 4):
        if sum(sizes) + s <= T:
            sizes.append(s)
    while sum(sizes) < T:
        sizes.append(min(8, T - sum(sizes)))
    groups, t0 = [], 0
    for s in sizes:
        groups.append((t0, s))
        t0 += s
    return groups


def _build_program(H: int, G: int, lay):
    T, CH = lay["T"], lay["CH"]
    chunk_of_tile = lay["chunk_of_tile"]
    first, last = lay["first"], lay["last"]
    assert H2 == 2 * H

    nc = bacc.Bacc("TRN2", target_bir_lowering=False, debug=False, num_devices=8)
    hl_d = nc.dram_tensor("feats_hl", [P, T * H2], F16, kind="ExternalInput")
    sl_d = nc.dram_tensor("aux_sl", [P, T], F32, kind="ExternalInput")
    rc_d = nc.dram_tensor("aux_rc", [P, CH], F32, kind="ExternalInput")
    iota_d = nc.dram_tensor("iota", [P, P], F16, kind="ExternalInput")
    out_d = nc.dram_tensor("out", [G, H], F32, kind="ExternalOutput")

    with tile.TileContext(nc) as tc:
        with (
            tc.tile_pool(name="const", bufs=1) as constp,
            tc.tile_pool(name="feats", bufs=4) as fpool,
            tc.tile_pool(name="mt", bufs=8) as mtpool,
            tc.tile_pool(name="outp", bufs=4) as opool,
            tc.tile_pool(name="psum", bufs=1, space="PSUM") as pp,
        ):
            # consts on the Activation HWDGE ring; feats keep SP to themselves
            iota_t = constp.tile([P, P], F16, tag="iota")
            nc.scalar.dma_start(iota_t[:], iota_d.ap())
            sl_t = constp.tile([P, T], F32, tag="sl")
            nc.scalar.dma_start(sl_t[:], sl_d.ap())
            rc_t = constp.tile([P, CH], F32, tag="rc")
            nc.scalar.dma_start(rc_t[:], rc_d.ap())

            psum_tiles = [
                pp.tile([P, H2], F32, tag=f"ps{c}", name=f"ps{c}") for c in range(CH)
            ]

            # PE HAM warmup: dummy matmuls on a zeroed tile while the first
            # feats DMA is in flight. psum[CH-1] is overwritten by its first
            # real matmul (start=True).
            dummy = constp.tile([P, H2], F16, tag="dummy")
            nc.vector.memset(dummy[:], 0.0)
            for _ in range(NDUMMY):
                nc.tensor.matmul(
                    psum_tiles[CH - 1][:], dummy[:, :P], dummy[:],
                    start=True, stop=True,
                )

            def finish(c):
                st = opool.tile([P, H], F32, tag="st", name="st")
                ot = opool.tile([P, H], F32, tag="ot", name="ot")
                if first[c] >= 0:
                    # st = psum_hi * (1/count) on ACT (single PSUM operand)
                    nc.scalar.activation(
                        st[:], psum_tiles[c][:, :H],
                        mybir.ActivationFunctionType.Copy,
                        scale=rc_t[:, c:c + 1],
                    )
                    # ot = (psum_lo * (1/count)) + st on DVE
                    nc.vector.scalar_tensor_tensor(
                        ot[:], psum_tiles[c][:, H:], rc_t[:, c:c + 1], st[:],
                        mybir.AluOpType.mult, mybir.AluOpType.add,
                    )
                else:
                    nc.vector.memset(ot[:], 0.0)
                # out DMA on the Activation ring, right behind chunk c's ACT
                nc.scalar.dma_start(out_d.ap()[c * P:(c + 1) * P, :], ot[:])

            for t0, nt in _dma_groups(T):
                ft = fpool.tile([P, 8 * H2], F16, tag="ft")
                nc.sync.dma_start(
                    ft[:, :nt * H2], hl_d.ap()[:, t0 * H2:(t0 + nt) * H2])
                for tt in range(nt):
                    t = t0 + tt
                    c = int(chunk_of_tile[t])
                    mt = mtpool.tile([P, P], F16, tag="mt", name="mt")
                    # onehot[s, g] = (iota[g] == sl[s]) on DVE
                    nc.vector.tensor_scalar(
                        mt[:],
                        iota_t[:],
                        sl_t[:, t:t + 1],
                        None,
                        mybir.AluOpType.is_equal,
                    )
                    nc.tensor.matmul(
                        psum_tiles[c][:], mt[:], ft[:, tt * H2:(tt + 1) * H2],
                        start=(t == first[c]), stop=(t == last[c]),
                    )
                    if t == last[c]:
                        finish(c)
            for c in range(CH):
                if first[c] < 0:
                    finish(c)

    nc.compile()
    return nc


def kernel(feats, segment_ids, num_groups, _trace=False):
    feats = np.ascontiguousarray(np.asarray(feats, dtype=np.float32))
    seg_all = np.ascontiguousarray(np.asarray(segment_ids, dtype=np.int32))
    G = int(num_groups)
    B, S, H = feats.shape
    assert seg_all.shape == (B, S) and B == 8 and G % P == 0

    lay = _host_layout(seg_all, G)
    T = lay["T"]
    nc = _build_program(H, G, lay)

    iota_arr = np.broadcast_to(
        np.arange(P, dtype=np.float16)[None, :], (P, P)
    ).copy()

    in_maps = []
    for r in range(B):
        fr = feats[r][lay["gather"][r]]  # [Spad, H] fp32, chunk-aligned
        hi = fr.astype(np.float16)
        lo = (fr - hi.astype(np.float32)).astype(np.float16)
        hl = np.concatenate([hi, lo], axis=1)  # [Spad, 2H]
        # transpose to [128, T*2H]: partition p, col t*2H+h <- row t*128+p
        hl_t = np.ascontiguousarray(
            hl.reshape(T, P, H2).transpose(1, 0, 2).reshape(P, T * H2))
        in_maps.append({
            "feats_hl": hl_t,
            "aux_sl": lay["aux_sl"][r],
            "aux_rc": lay["aux_rc"][r],
            "iota": iota_arr,
        })
    res = run_bass_kernel_spmd(nc, in_maps, list(range(B)), trace=_trace)
    out = np.stack([res.results[r]["out"] for r in range(B)])
    if _trace:
        return out, res
    return out


# revision 12
# speedup vs baseline: 1.1477x; 1.0650x over previous
"""Segment-mean reduction (grouped mean over sorted segment ids) on 8 trn2 cores.

Strategy (data-parallel over batch): each core handles one batch row.
out[g, :] = mean over rows s of feats with segment_ids[s] == g.

Host-side staging (inside kernel(), before upload):
  * The 1024 groups are split into 8 chunks of 128 groups. Rows of feats are
    reordered per core so that each chunk's rows are contiguous and padded to a
    multiple of 128; every 128-row tile then belongs to exactly ONE chunk, and
    the tile->chunk map is identical across cores (chunk tile counts are the
    max over cores). Pad rows point at row 0 with local id -1 (never matches).
  * feats are shipped as an fp16 hi/lo split (hi = fp16(x), lo = fp16(x - hi)),
    packed per tile as [128, 512] = [hi || lo]; adding the hi and lo halves of
    the 512-wide matmul output recovers ~fp32 accuracy with ONE matmul/tile.
  * DRAM layout is transposed to [128, T*512] so every DMA partition line is a
    large contiguous run (1 KiB per tile per partition; tiles of one group are
    adjacent) instead of the 1 KiB strided gather of the [T*128, 512] layout.

Device program (static schedule):
  * consts (iota / sl / rc) stream in on the Activation HWDGE ring; feats
    groups stream in on the SP ring (skewed sizes: small first groups so the
    first matmul starts early, then 8-tile groups).
  * ~10 dummy matmuls on garbage SBUF warm the PE HAM clock gate (cold PE runs
    at 1.2 GHz; warm at 2.4 GHz) during the engine-bringup/first-DMA window.
    They write psum[7], which the first real accumulation overwrites
    (start=True).
  * per tile t: onehot[s, g] = (iota[g] == sl[s]) on DVE (all-fp16), then
    psum[chunk(t)][:, 0:512] += onehot.T @ (hi || lo) on PE.
  * per chunk c, emitted right after chunk c's last tile: st = psum_hi * rc on
    ACT, ot = psum_lo * rc + st on DVE, then out-row DMA on the Activation
    ring (so it never queues behind feats triggers on SP).

Accumulation order per chunk, the hi/lo split, and the finish math are
IDENTICAL to the reference baseline kernel => bitwise-equal outputs.

Per-core HBM traffic ~= feats (9.4 MB padded) + out (1 MB) => memory-bound;
roofline ~29 us at the ~358 GB/s per-core HBM limit.
"""

import numpy as np

import concourse.bass as bass
import concourse.bacc as bacc
import concourse.mybir as mybir
import concourse.tile as tile
from concourse.bass_utils import run_bass_kernel_spmd

F32 = mybir.dt.float32
F16 = mybir.dt.float16
P = 128  # partitions
H2 = 512  # hi || lo columns per tile
NDUMMY = 8  # PE warmup matmuls


def _host_layout(seg_all: np.ndarray, G: int):
    """Chunk-aligned row layout: shared tile->chunk map, per-core gather
    indices and aux arrays."""
    R, S = seg_all.shape
    CH = G // P

    chunk_of = seg_all // P  # [R, S]
    cnt = np.stack([np.bincount(chunk_of[r], minlength=CH) for r in range(R)])
    tiles_per_chunk = (cnt.max(axis=0) + P - 1) // P  # [CH]
    T = int(tiles_per_chunk.sum())

    chunk_of_tile = np.repeat(np.arange(CH), tiles_per_chunk)  # [T]
    first = np.full(CH, -1, np.int64)
    last = np.full(CH, -1, np.int64)
    for i, c in enumerate(chunk_of_tile):
        if first[c] < 0:
            first[c] = i
        last[c] = i

    Spad = T * P
    gather = np.zeros((R, Spad), np.int64)
    sl = np.full((R, Spad), -1.0, np.float32)  # local group id, -1 for pads
    for r in range(R):
        pos = 0
        for c in range(CH):
            rows = np.nonzero(chunk_of[r] == c)[0]
            n = len(rows)
            tc = int(tiles_per_chunk[c])
            gather[r, pos:pos + n] = rows
            sl[r, pos:pos + n] = (seg_all[r, rows] % P).astype(np.float32)
            pos += tc * P
    # aux arrays in [P, T] tile layout: column t, partition p <- padded row t*P+p
    aux_sl = np.ascontiguousarray(sl.reshape(R, T, P).transpose(0, 2, 1))
    # per-group reciprocal counts, [P, CH]: partition p, col c -> group c*P+p
    counts = np.stack(
        [np.bincount(seg_all[r], minlength=G) for r in range(R)]
    ).astype(np.float32)
    recip = (1.0 / np.maximum(counts, 1.0)).reshape(R, CH, P)
    aux_rc = np.ascontiguousarray(recip.transpose(0, 2, 1))

    return dict(T=T, CH=CH, chunk_of_tile=chunk_of_tile, first=first, last=last,
                gather=gather, aux_sl=aux_sl, aux_rc=aux_rc)


def _dma_groups(T: int):
    """Skewed group sizes: small first so the first matmul starts early."""
    sizes = []
    for s in (2, 2, 4):
        if sum(sizes) + s <= T:
            sizes.append(s)
    while sum(sizes) < T:
        sizes.append(min(8, T - sum(sizes)))
    groups, t0 = [], 0
    for s in sizes:
        groups.append((t0, s))
        t0 += s
    return groups


def _build_program(H: int, G: int, lay):
    T, CH = lay["T"], lay["CH"]
    chunk_of_tile = lay["chunk_of_tile"]
    first, last = lay["first"], lay["last"]
    assert H2 == 2 * H

    nc = bacc.Bacc("TRN2", target_bir_lowering=False, debug=False, num_devices=8)
    hl_d = nc.dram_tensor("feats_hl", [P, T * H2], F16, kind="ExternalInput")
    sl_d = nc.dram_tensor("aux_sl", [P, T], F32, kind="ExternalInput")
    rc_d = nc.dram_tensor("aux_rc", [P, CH], F32, kind="ExternalInput")
    out_d = nc.dram_tensor("out", [G, H], F32, kind="ExternalOutput")

    with tile.TileContext(nc) as tc:
        with (
            tc.tile_pool(name="const", bufs=1) as constp,
            tc.tile_pool(name="feats", bufs=6) as fpool,
            tc.tile_pool(name="mt", bufs=8) as mtpool,
            tc.tile_pool(name="outp", bufs=4) as opool,
            tc.tile_pool(name="psum", bufs=1, space="PSUM") as pp,
        ):
            # sl rides the SP ring ahead of the feats groups (SP comes up
            # first); rc rides the Activation ring (not needed until the
            # first chunk finish); iota is generated on-chip.
            sl_t = constp.tile([P, T], F32, tag="sl")
            nc.sync.dma_start(sl_t[:], sl_d.ap())
            rc_t = constp.tile([P, CH], F32, tag="rc")
            nc.scalar.dma_start(rc_t[:], rc_d.ap())
            iota_t = constp.tile([P, P], F16, tag="iota")
            nc.gpsimd.iota(
                iota_t[:], [[1, P]], base=0, channel_multiplier=0,
                allow_small_or_imprecise_dtypes=True,
            )

            psum_tiles = [
                pp.tile([P, H2], F32, tag=f"ps{c}", name=f"ps{c}") for c in range(CH)
            ]

            # PE HAM warmup: dummy matmuls on a zeroed tile while the first
            # feats DMA is in flight. psum[CH-1] is overwritten by its first
            # real matmul (start=True).
            dummy = constp.tile([P, H2], F16, tag="dummy")
            nc.vector.memset(dummy[:], 0.0)
            for _ in range(NDUMMY):
                nc.tensor.matmul(
                    psum_tiles[CH - 1][:], dummy[:, :P], dummy[:],
                    start=True, stop=True,
                )

            def finish(c):
                st = opool.tile([P, H], F32, tag="st", name="st")
                ot = opool.tile([P, H], F32, tag="ot", name="ot")
                if first[c] >= 0:
                    # st = psum_hi * (1/count) on ACT (single PSUM operand)
                    nc.scalar.activation(
                        st[:], psum_tiles[c][:, :H],
                        mybir.ActivationFunctionType.Copy,
                        scale=rc_t[:, c:c + 1],
                    )
                    # ot = (psum_lo * (1/count)) + st on DVE
                    nc.vector.scalar_tensor_tensor(
                        ot[:], psum_tiles[c][:, H:], rc_t[:, c:c + 1], st[:],
                        mybir.AluOpType.mult, mybir.AluOpType.add,
                    )
                else:
                    nc.vector.memset(ot[:], 0.0)
                # out DMA on the Activation ring, right behind chunk c's ACT
                nc.scalar.dma_start(out_d.ap()[c * P:(c + 1) * P, :], ot[:])

            for t0, nt in _dma_groups(T):
                ft = fpool.tile([P, 8 * H2], F16, tag="ft")
                nc.sync.dma_start(
                    ft[:, :nt * H2], hl_d.ap()[:, t0 * H2:(t0 + nt) * H2])
                for tt in range(nt):
                    t = t0 + tt
                    c = int(chunk_of_tile[t])
                    mt = mtpool.tile([P, P], F16, tag="mt", name="mt")
                    # onehot[s, g] = (iota[g] == sl[s]) on DVE
                    nc.vector.tensor_scalar(
                        mt[:],
                        iota_t[:],
                        sl_t[:, t:t + 1],
                        None,
                        mybir.AluOpType.is_equal,
                    )
                    nc.tensor.matmul(
                        psum_tiles[c][:], mt[:], ft[:, tt * H2:(tt + 1) * H2],
                        start=(t == first[c]), stop=(t == last[c]),
                    )
                    if t == last[c]:
                        finish(c)
            for c in range(CH):
                if first[c] < 0:
                    finish(c)

    nc.compile()
    return nc


def kernel(feats, segment_ids, num_groups, _trace=False):
    feats = np.ascontiguousarray(np.asarray(feats, dtype=np.float32))
    seg_all = np.ascontiguousarray(np.asarray(segment_ids, dtype=np.int32))
    G = int(num_groups)
    B, S, H = feats.shape
    assert seg_all.shape == (B, S) and B == 8 and G % P == 0

    lay = _host_layout(seg_all, G)
    T = lay["T"]
    nc = _build_program(H, G, lay)

    in_maps = []
    for r in range(B):
        fr = feats[r][lay["gather"][r]]  # [Spad, H] fp32, chunk-aligned
        hi = fr.astype(np.float16)
        lo = (fr - hi.astype(np.float32)).astype(np.float16)
        hl = np.concatenate([hi, lo], axis=1)  # [Spad, 2H]
        # transpose to [128, T*2H]: partition p, col t*2H+h <- row t*128+p
        hl_t = np.ascontiguousarray(
            hl.reshape(T, P, H2).transpose(1, 0, 2).reshape(P, T * H2))
        in_maps.append({
            "feats_hl": hl_t,
            "aux_sl": lay["aux_sl"][r],
            "aux_rc": lay["aux_rc"][r],
        })
    res = run_bass_kernel_spmd(nc, in_maps, list(range(B)), trace=_trace)
    out = np.stack([res.results[r]["out"] for r in range(B)])
    if _trace:
        return out, res
    return out
